# revision 11
# baseline (speedup 1.0000x reference)
"""Trainium2 Bass kernel for a 2-layer GRU extractor.

Reference computes: 2-layer PyTorch-convention GRU (H=40) over x (B=4096,
T=256, I=16), returning layer-1 final hidden state (B, 40).

Data-parallel over 8 NeuronCores (512 batch rows each). Per core, batch-major
layout: 512 = 4 tiles of 128 batch rows on SBUF partitions, gates on the free
dim. Per step and layer, per batch tile (psum cols [rz 0:80 | hn 80:120 |
xn 120:160]):
  psum[:, i, 0:160]  = [x|1] @ [WihT-ext]   (start; zero weights in hn cols)
  psum[:, i, 0:120] += [h|1] @ [WhhT-ext]   (stop; rz accum + hn)
The z-gate weights are NEGATED so sigmoid yields z' = 1-z directly:
  rz' = sigmoid(psum rz);  n = tanh(xn + r*hn)
  h'  = z'*n + (h - z'*h)    (h - z'*h precomputed on GpSimd off-path)
h' (fp16) lands in a transpose-source buffer; a TensorE is_transpose matmul
(vs a 128x128 fp16 identity) transposes each batch-tile pair into spare bytes
of the gates' PSUM banks, and one copy per layer brings hT back to SBUF for
the next step's stationary operand. Ones-columns in the transpose source
regenerate the bias row of hT each step. Layer 1 is emitted one step behind
layer 0 so the two layers' chains overlap on the engines.
"""

import sys

sys.path.insert(0, "/opt/trn_rl_repo")

import numpy as np

B, T, I, H = 4096, 256, 16, 40
NCORES = 8
BL = B // NCORES  # 512 batch rows per core
G = 3 * H  # 120 stacked gate rows (r, z, n)

_CACHE = {}


def _apply_tile_patch():
    """This walrus build rejects >2 sync waits on one instruction. Split the
    TileContext tail drain's accumulated sem waits into one SP nop each."""
    import concourse.tile as tile_mod
    import concourse.mybir as mybir
    from concourse.vector_clock import ScopedClock

    def _drain_and_barrier(self, tick_clock, wait_clock):
        probe = self.nc.sync.nop()
        wait_clock.add_sem_waits(
            probe.ins, ScopedClock({None: tick_clock.global_clock})
        )
        waits = list(probe.ins.sync_info.on_wait)
        del probe.ins.sync_info.on_wait[:]
        if waits:
            probe.ins.sync_info.on_wait.append(waits[0])
        for w in waits[1:]:
            n2 = self.nc.sync.nop()
            if n2.ins.sync_info is None:
                n2.ins.sync_info = mybir.SyncInfo(on_wait=[], on_update=[])
            n2.ins.sync_info.on_wait.append(w)
        self.nc.sync.drain()
        self.nc.all_engine_barrier()
        assert self.sems is not None
        popped = self.nc._tile_sem_poison_stack.pop()
        assert popped is self._sem_poison
        self.nc.clear_and_free_semaphores(list(self.sems.allocated().values()))
        self.nc.all_engine_barrier()

    tile_mod.TileContext._drain_and_barrier = _drain_and_barrier


def _build(n_steps):
    import concourse.bass as bass
    import concourse.mybir as mybir
    import concourse.tile as tile
    from concourse import masks
    from concourse.tile_rust import add_dep_helper

    _apply_tile_patch()

    f16 = mybir.dt.float16
    f32 = mybir.dt.float32
    AF = mybir.ActivationFunctionType
    OP = mybir.AluOpType

    nc = bass.Bass()
    x_ext = nc.declare_dram_parameter("xsb", [128, n_steps, 128], f16, isOutput=False)
    wh0_ext = nc.declare_dram_parameter("wh0", [128, 120], f16, isOutput=False)
    wx0_ext = nc.declare_dram_parameter("wx0", [128, 120], f16, isOutput=False)
    wh1_ext = nc.declare_dram_parameter("wh1", [128, 120], f16, isOutput=False)
    wx1_ext = nc.declare_dram_parameter("wx1", [128, 120], f16, isOutput=False)
    out_ext = nc.declare_dram_parameter("out", [BL, H], f32, isOutput=True)

    with tile.TileContext(nc) as tc:
        with (
            tc.tile_pool(name="const", bufs=1) as cpool,
            tc.tile_pool(name="gates", bufs=3) as gpool,
            tc.tile_pool(name="psum", bufs=1, space="PSUM") as ppool,
        ):
            xsb = cpool.tile([128, n_steps, 128], f16)
            wh = [cpool.tile([128, 120], f16, name=f"wh{l}") for l in range(2)]
            wx = [cpool.tile([128, 120], f16, name=f"wx{l}") for l in range(2)]
            ident = cpool.tile([128, 128], f16, name="ident")
            # hT[l]: transposed state, block p covers batch tiles 2p (rows
            # 0:41 incl ones row) and 2p+1 (rows 64:105).
            hT = [cpool.tile([128, 2, 128], f16, name=f"hT{l}") for l in range(2)]
            # hsrc[l]: B-major state, tile i at [:, i, 0:40]; col 40 = 1.0
            # (becomes hT's ones row through the transpose).
            hsrc = [cpool.tile([128, 4, 64], f16, name=f"hsrc{l}") for l in range(2)]
            # gates psum: tile i occupies bank i cols 0:160; cols 256:320
            # (bytes 1024:1280 of banks 0,1) hold the hT transposes.
            psum = [ppool.tile([128, 4, 512], f32, name=f"psum{l}") for l in range(2)]

            nc.sync.dma_start(out=xsb[:], in_=x_ext[:])
            for l, ext in ((0, wh0_ext), (1, wh1_ext)):
                nc.sync.dma_start(out=wh[l][:], in_=ext[:])
            for l, ext in ((0, wx0_ext), (1, wx1_ext)):
                nc.sync.dma_start(out=wx[l][:], in_=ext[:])

            masks.make_identity(nc, ident[:])
            for l in range(2):
                nc.vector.memset(hsrc[l][:], 0.0)
                nc.vector.memset(hsrc[l][:, :, 40:41], 1.0)

            prev_copy = [None, None]

            def emit_mms(l, t):
                # all m_x first, then all m_h: adjacent instructions hit
                # different PSUM banks so the PE pipeline overlaps them
                # (same-bank back-to-back pays a ~200ns pipe turnaround).
                ps = psum[l]
                m1s = []
                for i in range(4):
                    blk, pos = i // 2, 64 * (i % 2)
                    if l == 0:
                        xpos = 32 * i
                        lhsT_x = xsb[xpos : xpos + 17, t, :]
                        wxa = wx[0][xpos : xpos + 17, :]
                        tp_x = (xpos, 0)
                    else:
                        lhsT_x = hT[0][pos : pos + 41, blk, :]
                        wxa = wx[1][pos : pos + 41, :]
                        tp_x = (pos, 0)
                    m1 = nc.tensor.matmul(
                        ps[:, i, 0:120],
                        lhsT_x,
                        wxa,
                        start=True,
                        stop=False,
                        tile_position=tp_x,
                    )
                    m1s.append(m1)
                    # start=True zeroes the whole 2KB bank; banks 0,1 hold the
                    # previous step's hT bytes until the copy drains them.
                    if i < 2 and prev_copy[l] is not None:
                        add_dep_helper(
                            m1.ins,
                            prev_copy[l].ins,
                            reason="m_x zeroes bank holding prev hT",
                        )
                for i in range(4):
                    blk, pos = i // 2, 64 * (i % 2)
                    m2 = nc.tensor.matmul(
                        ps[:, i, 40:160],
                        hT[l][pos : pos + 41, blk, :],
                        wh[l][pos : pos + 41, :],
                        start=False,
                        stop=True,
                        tile_position=(pos, 0),
                    )
                    # has_written bit protocol: start=True must run first.
                    add_dep_helper(m2.ins, m1s[i].ins, sync=False)

            def sigma(l):
                rz = gpool.tile([128, 4, 80], f16, tag=f"rz{l}")
                nc.scalar.activation(rz[:], psum[l][:, :, 40:120], AF.Sigmoid)
                return rz

            def emit_tail(l):
                ps = psum[l]
                for p in range(2):
                    nc.tensor.matmul(
                        ps[:, p, 256:320].bitcast(f16),
                        hsrc[l][:, 2 * p : 2 * p + 2, :],
                        ident[:],
                        is_transpose=True,
                    )
                src = ps[:, 0:2, 256:320].bitcast(f16)
                if l == 0:
                    prev_copy[l] = nc.vector.tensor_copy(hT[l][:, 0:2, :], src)
                else:
                    # ACT is idle at tick end; keeps the copy off the DVE
                    # queue where it would delay l0's h' update.
                    prev_copy[l] = nc.scalar.copy(hT[l][:, 0:2, :], src)

            # initial hT (h=0 + ones rows) via the same transpose+copy path
            for l in range(2):
                emit_tail(l)

            # Per tick u: layer 1 runs step u-1 while layer 0 runs step u, so
            # the two serial chains overlap. Ops are interleaved across layers
            # per engine in readiness order to avoid head-of-line stalls.
            h_ap = [hsrc[l][:, :, 0:40] for l in range(2)]
            for u in range(n_steps + 1):
                # active (layer, ...) list: l1 first (one pipeline step ahead)
                act = []
                if u >= 1:
                    act.append(1)
                if u < n_steps:
                    act.append(0)
                for l in act:
                    emit_mms(l, u - 1 if l == 1 else u)
                rz = {l: sigma(l) for l in act}
                t2 = {}
                for l in act:
                    t2[l] = gpool.tile([128, 4, 40], f16, tag=f"t2{l}", name=f"t2{l}")
                    nc.vector.tensor_tensor(
                        t2[l][:], rz[l][:, :, 0:40], psum[l][:, :, 120:160], op=OP.mult
                    )
                t3 = {}
                for l in act:
                    t3[l] = gpool.tile([128, 4, 40], f16, tag=f"t3{l}", name=f"t3{l}")
                    nc.vector.tensor_tensor(
                        t3[l][:], t2[l][:], psum[l][:, :, 0:40], op=OP.add
                    )
                nt = {}
                for l in act:
                    nt[l] = gpool.tile([128, 4, 40], f16, tag=f"nt{l}", name=f"nt{l}")
                    nc.scalar.activation(nt[l][:], t3[l][:], AF.Tanh)
                # h' = h + z'*(n - h); all fp16 SBUF ops — GpSimd is kept idle
                # (it shares the DVE SBUF port; concurrent GpSimd TTs were
                # doubling the DVE chain ops' latency).
                for l in act:
                    dd = gpool.tile([128, 4, 40], f16, tag=f"d{l}")
                    nc.vector.tensor_tensor(dd[:], nt[l][:], h_ap[l], op=OP.subtract)
                    q = gpool.tile([128, 4, 40], f16, tag=f"q{l}")
                    nc.vector.tensor_tensor(
                        q[:], rz[l][:, :, 40:80], dd[:], op=OP.mult
                    )
                    nc.vector.tensor_tensor(h_ap[l], h_ap[l], q[:], op=OP.add)
                for l in act:
                    if l == 1 and u >= n_steps:
                        continue  # layer 1's final state needs no transpose
                    emit_tail(l)

            hout = cpool.tile([128, 4, 40], f32)
            nc.vector.tensor_copy(hout[:], hsrc[1][:, :, 0:40])
            for i in range(4):
                nc.sync.dma_start(
                    out=out_ext[i * 128 : (i + 1) * 128, :], in_=hout[:, i, :]
                )
    _split_excess_waits(nc, mybir)
    return nc


def _split_excess_waits(nc, mybir, limit=1):
    """walrus CoreV3 rejects instructions with several sync waits. Move all
    but `limit` waits of any instruction onto fresh NOPs inserted just before
    it on the same engine."""
    for fn in nc.m.functions:
        for bb in fn.blocks:
            insts = bb.instructions
            new_list = []
            for inst in insts:
                si = getattr(inst, 'sync_info', None)
                if si is not None and si.on_wait is not None and len(si.on_wait) > limit:
                    waits = list(si.on_wait)
                    del si.on_wait[:]
                    si.on_wait.extend(waits[-limit:])
                    for w in waits[:-limit]:
                        nop = mybir.InstNoOp(
                            name=nc.get_next_instruction_name(),
                            ins=[],
                            outs=[],
                            engine=inst.engine,
                            sync_info=mybir.SyncInfo(on_wait=[w], on_update=[]),
                        )
                        new_list.append(nop)
                new_list.append(inst)
            del insts[:]
            insts.extend(new_list)


def _prep_inputs(x, Wih0, Whh0, bih0, bhh0, Wih1, Whh1, bih1, bhh1, n_steps):
    x = np.asarray(x, np.float32)
    f = lambda a: np.asarray(a, np.float32)
    Wih0, Whh0, bih0, bhh0 = map(f, (Wih0, Whh0, bih0, bhh0))
    Wih1, Whh1, bih1, bhh1 = map(f, (Wih1, Whh1, bih1, bhh1))

    def neg_z(w):
        # negate the z-gate rows (40:80 of the stacked (r,z,n) dim)
        w = w.copy()
        w[40:80] = -w[40:80]
        return w

    def h_ext(Whh, bih, bhh):
        # (41, 120): WhhT for [r|z|n] with bias row: rz = bih+bhh, n = bhh
        W = neg_z(Whh)
        b = neg_z(bih + bhh).copy()
        b[80:120] = bhh[80:120]
        return np.concatenate([W.T, b[None, :]], axis=0).astype(np.float16)

    def x_ext(Wih, bih):
        # (K+1, 120): [Win | Wir | -Wiz] with bias row [bih_n | 0 | 0]
        W = neg_z(Wih)
        K = W.shape[1]
        out = np.zeros((K + 1, 120), np.float32)
        out[:K, 0:40] = W[80:120].T
        out[:K, 40:120] = W[0:80].T
        out[K, 0:40] = bih[80:120]
        return out.astype(np.float16)

    wh0_e = h_ext(Whh0, bih0, bhh0)  # (41, 120)
    wx0_e = x_ext(Wih0, bih0)  # (17, 120)
    wh1_e = h_ext(Whh1, bih1, bhh1)  # (41, 120)
    wx1_e = x_ext(Wih1, bih1)  # (41, 120)

    wh0 = np.zeros((128, 120), np.float16)
    wh0[0:41] = wh0_e
    wh0[64:105] = wh0_e
    wh1 = np.zeros((128, 120), np.float16)
    wh1[0:41] = wh1_e
    wh1[64:105] = wh1_e
    wx1 = np.zeros((128, 120), np.float16)
    wx1[0:41] = wx1_e
    wx1[64:105] = wx1_e
    wx0 = np.zeros((128, 120), np.float16)
    for i in range(4):
        wx0[32 * i : 32 * i + 17] = wx0_e

    # xsb per core: (128, T, 128); rows 32i:32i+16 = features of batch tile i,
    # row 32i+16 = 1.0 (bias ones row for the x-side projection).
    xc = x[:, :n_steps, :].reshape(NCORES, 4, 128, n_steps, I)
    xc = np.ascontiguousarray(xc.transpose(0, 1, 4, 3, 2))  # (8,4,16,T,128)
    xsb = np.zeros((NCORES, 128, n_steps, 128), np.float16)
    for i in range(4):
        xsb[:, 32 * i : 32 * i + 16] = xc[:, i]
        xsb[:, 32 * i + 16] = 1.0
    return xsb, wh0, wx0, wh1, wx1


def kernel(x, Wih0, Whh0, bih0, bhh0, Wih1, Whh1, bih1, bhh1):
    from concourse.bass_utils import run_bass_kernel_spmd

    n_steps = T
    if "nc" not in _CACHE:
        _CACHE["nc"] = _build(n_steps)
    nc = _CACHE["nc"]

    xsb, wh0, wx0, wh1, wx1 = _prep_inputs(
        x, Wih0, Whh0, bih0, bhh0, Wih1, Whh1, bih1, bhh1, n_steps
    )
    in_maps = [
        {"xsb": xsb[c], "wh0": wh0, "wx0": wx0, "wh1": wh1, "wx1": wx1}
        for c in range(NCORES)
    ]
    res = run_bass_kernel_spmd(nc, in_maps, list(range(NCORES)))
    out = np.concatenate(
        [np.asarray(res.results[c]["out"]) for c in range(NCORES)], axis=0
    )
    return out.astype(np.float32)
